# revision 17
# baseline (speedup 1.0000x reference)
"""Trainium2 Bass kernel for ComplementConstraintCombined.

Computes, for full inputs x[8192,2048], W[2048,1000], b[1000]:
    out = x @ W + b
    lse = logsumexp(out, axis=1, keepdims=True)
    return out - (lse + log1p(-exp(out - lse)))

Math rewrite used on-device (per row):
    t = exp(out); s = sum_c t
    result = out - ln(s - t)        # == out - lse - log1p(-exp(out-lse))
(no cancellation risk: max softmax prob here is ~0.03)

Strategy:
  - Data-parallel over batch: 1024 rows per core, W/b replicated.
  - Host pre-transposes x and quantizes x,W to fp8-e4m3; the bias is
    folded in as an extra contraction row (x'=1/16 exact, W'=16*b).
  - Device runs fp8 DoubleRow matmuls (2 k-subtiles, 0.5 cyc/row) with
    one explicit ldweights per stationary tile reused by 4 C-chunks.
  - Epilogue: ACT exp(+accum) from PSUM -> bf16 t; DVE u = s - t (bf16,
    2x mode); ACT ln(u); DVE res = psum - g -> bf16; DMA out bf16.
  - Host upcasts the bf16 result to fp32.
"""
import sys

sys.path.insert(0, "/opt/trn_rl_repo")

import ml_dtypes
import numpy as np

import concourse.bass as bass
import concourse.mybir as mybir
from concourse.bass_utils import run_bass_kernel_spmd
from concourse.tile import TileContext

B, D, C = 8192, 2048, 1000
NCORES = 8
BS = B // NCORES      # 1024 rows per core
P = 128               # partitions
KO = 17               # k-subtiles: 1 bias row subtile + 16 data subtiles
KP = 8                # DoubleRow k-pairs over the 16 data subtiles
KPAD = KO * P         # 2176 padded contraction dim
MT = BS // P          # 8 m-tiles per core
CH = 250              # matmul free-dim chunk (DoubleRow moving limit 512 = 2*CH+pad)
NCH = C // CH         # 4 chunks
BIAS_X = 1.0 / 16.0   # ones-column value (exact in e4m3)
W_SCALE = 64.0        # lifts W out of fp8-subnormal range (HW flushes denormals)
INV_W_SCALE = 1.0 / W_SCALE
N_WARM = 28
F = mybir.dt.float32
F8 = mybir.dt.float8e4
BF = mybir.dt.bfloat16
AF = mybir.ActivationFunctionType
ALU = mybir.AluOpType
DR = mybir.MatmulPerfMode.DoubleRow

E4NP = ml_dtypes.float8_e4m3
BFNP = ml_dtypes.bfloat16


def _split_multi_waits(nc, max_waits=1):
    """walrus codegen on this toolchain allows a single sync-wait command per
    instruction; hoist extra waits into standalone NOPs on the same engine."""
    n = 0
    for fn in nc.m.functions:
        for bb in fn.blocks:
            new = []
            for inst in bb.instructions:
                si = inst.sync_info
                if si is not None and len(si.on_wait) > max_waits:
                    waits = list(si.on_wait)
                    for j, w in enumerate(waits[:-max_waits]):
                        nop = mybir.InstNoOp(
                            name=f"{inst.name}-w{j}", engine=inst.engine
                        )
                        nop.sync_info = mybir.SyncInfo(on_wait=[w], on_update=[])
                        new.append(nop)
                        n += 1
                    inst.sync_info = mybir.SyncInfo(
                        on_wait=waits[-max_waits:], on_update=list(si.on_update)
                    )
                new.append(inst)
            bb.instructions = new
    return n


def _body(nc, tc, xt, w, out, ctx):
    wx = ctx.enter_context(tc.tile_pool(name="wx", bufs=1))
    work = ctx.enter_context(tc.tile_pool(name="work", bufs=4))
    pso = ctx.enter_context(tc.tile_pool(name="pso", bufs=4, space="PSUM"))

    xt3 = xt.rearrange("(ko p) m -> p ko m", p=P)
    w3 = w.rearrange("(ko p) c -> p ko c", p=P)
    out2 = out.rearrange("(mt p) c -> mt p c", p=P)

    xt_sb = wx.tile([P, KO, BS], F8)
    w_sb = wx.tile([P, KO, C], F8)

    # Input strips: each DMA trigger costs ~0.7us on its issuing engine, so
    # batch into 4 ascending-size chunks per tensor and spread the triggers
    # over four otherwise-idle engines; the first chunk unblocks matmul kp0.
    DMA_SPLITS = [(0, 3), (3, 7), (7, 13), (13, KO)]
    XT_ENG = [nc.gpsimd, nc.scalar, nc.gpsimd, nc.scalar]
    W_ENG = [nc.sync, nc.gpsimd, nc.sync, nc.scalar]
    for i, (lo, hi) in enumerate(DMA_SPLITS):
        XT_ENG[i].dma_start(xt_sb[:, lo:hi, :], xt3[:, lo:hi, :])
        W_ENG[i].dma_start(w_sb[:, lo:hi, :], w3[:, lo:hi, :])

    # PE clock warmup on a zeroed fp8 tile while the DMAs land. Warmup
    # output goes into m-tile 0's own (not-yet-started) PSUM tile so all
    # four ring slots stay available for real accumulations.
    wtile = work.tile([P, 2, P], F8, tag="warm")
    nc.vector.memset(wtile, 0)

    ps_tiles = {}

    def bias_opener(mt):
        # Open each accumulation group with a plain-mode matmul on the bias
        # subtile: the first start=True DoubleRow write into a fresh PSUM
        # bank drops its first k-subtile on this hardware, so the group
        # opener must not be a DoubleRow op.
        msl = slice(mt * P, (mt + 1) * P)
        if mt not in ps_tiles:
            ps_tiles[mt] = pso.tile([P, 2, 512], F, tag="ps", name=f"ps_{mt}")
        ps = ps_tiles[mt]
        for ch in range(NCH):
            bk, co = divmod(ch, 2)
            nc.tensor.matmul(
                ps[:, bk, co * CH:(co + 1) * CH],
                xt_sb[:, 0, msl],
                w_sb[:, 0, ch * CH:(ch + 1) * CH],
                start=True,
                stop=False,
            )

    def kp_step(mt, kp):
        msl = slice(mt * P, (mt + 1) * P)
        ps = ps_tiles[mt]
        ksl = slice(2 * kp + 1, 2 * kp + 3)
        lhsT = xt_sb[:, ksl, msl]
        nc.tensor.ldweights(lhsT, perf_mode=DR)
        for ch in range(NCH):
            bk, co = divmod(ch, 2)
            mm = nc.tensor.matmul(
                ps[:, bk, co * CH:(co + 1) * CH],
                lhsT,
                w_sb[:, ksl, ch * CH:(ch + 1) * CH],
                start=False,
                stop=(kp == KP - 1),
                perf_mode=DR,
            )
            mm.ins.ldweights = False

    def epilogue(mt):
        # res = o - ln(s - exp(o)), all views chunk-matched
        ps = ps_tiles[mt]
        ps_v = ps[:, :, 0:2 * CH]                        # [P, 2, 500] fp32
        t = work.tile([P, C], BF, tag="t", name=f"t_{mt}")
        t_v = t[:, :].rearrange("p (b c) -> p b c", b=2)
        s = work.tile([P, 1], F, tag="s", name=f"s_{mt}")
        nc.scalar.activation(t_v, ps_v, AF.Exp, scale=INV_W_SCALE, accum_out=s)
        u = work.tile([P, C], BF, tag="u", name=f"u_{mt}")
        nc.vector.tensor_scalar(u, t, s[:, :], -1.0, ALU.subtract, ALU.mult)
        g = work.tile([P, C], F, tag="g", name=f"g_{mt}")
        nc.scalar.activation(g, u, AF.Ln)
        res = work.tile([P, C], BF, tag="res", name=f"res_{mt}")
        res_v = res[:, :].rearrange("p (b c) -> p b c", b=2)
        g_v = g[:, :].rearrange("p (b c) -> p b c", b=2)
        nc.vector.scalar_tensor_tensor(
            res_v, ps_v, INV_W_SCALE, g_v, ALU.mult, ALU.subtract
        )
        nc.sync.dma_start(out2[mt], res)

    # Warmup matmuls into mt0's future tile (results discarded: the bias
    # opener below rewrites it with start=True).
    ps0 = ps_tiles[0] = pso.tile([P, 2, 512], F, tag="ps", name="ps_0")
    for _ in range(N_WARM):
        nc.tensor.matmul(
            ps0[:, 0, 0:P], wtile, wtile, start=True, stop=True, perf_mode=DR
        )

    # m-tiles 0-2: chunk-staged kp-major so PE always has DMA-ready work
    # while the input chunks land; staggers their completions only slightly,
    # so their epilogues interleave with m-tiles 3-7 below. Using 3 tiles
    # here keeps one PSUM ring slot free, so m-tile 3 starts stall-free.
    GROUP_A = range(3)
    for mt in GROUP_A:
        bias_opener(mt)
        kp_step(mt, 0)
    for mt in GROUP_A:
        for kp in (1, 2):
            kp_step(mt, kp)
    for mt in GROUP_A:
        for kp in (3, 4, 5):
            kp_step(mt, kp)
    for mt in GROUP_A:
        kp_step(mt, 6)
        kp_step(mt, 7)
        epilogue(mt)

    # m-tiles 3-7: all inputs resident by now; straight per-tile pipeline.
    for mt in range(3, MT):
        bias_opener(mt)
        for kp in range(KP):
            kp_step(mt, kp)
        epilogue(mt)


_NC = None


def _build():
    global _NC
    if _NC is not None:
        return _NC
    nc = bass.Bass()
    xt = nc.declare_dram_parameter("xt", [KPAD, BS], F8, isOutput=False)
    w = nc.declare_dram_parameter("w", [KPAD, C], F8, isOutput=False)
    out = nc.declare_dram_parameter("out", [BS, C], BF, isOutput=True)
    from contextlib import ExitStack

    with TileContext(nc) as tc, ExitStack() as ctx:
        _body(nc, tc, xt[:, :], w[:, :], out[:, :], ctx)
    _split_multi_waits(nc)
    _NC = nc
    return nc


def kernel(x, W, b, trace=False):
    x = np.asarray(x, dtype=np.float32)
    W = np.asarray(W, dtype=np.float32)
    b = np.asarray(b, dtype=np.float32)

    # Host-side prep (not on the device critical path): transpose + fp8
    # quantize x, quantize W, fold bias in as one extra contraction row.
    xT8 = np.empty((KPAD, B), dtype=E4NP)
    xT8[0] = E4NP(BIAS_X)
    xT8[P:P + D] = np.ascontiguousarray(x.astype(E4NP).T)
    xT8[1:P] = E4NP(0.0)
    W8 = np.empty((KPAD, C), dtype=E4NP)
    W8[0] = (b * (W_SCALE / BIAS_X)).astype(E4NP)
    W8[1:P] = E4NP(0.0)
    W8[P:P + D] = (W * W_SCALE).astype(E4NP)

    nc = _build()
    in_maps = [
        {"xt": np.ascontiguousarray(xT8[:, i * BS:(i + 1) * BS]), "w": W8}
        for i in range(NCORES)
    ]
    r = run_bass_kernel_spmd(nc, in_maps, list(range(NCORES)), trace=trace)
    outp = np.concatenate(
        [r.results[i]["out"].astype(np.float32) for i in range(NCORES)], axis=0
    )
    if trace:
        return outp, r
    return outp


# revision 18
# speedup vs baseline: 1.0480x; 1.0480x over previous
"""Trainium2 Bass kernel for ComplementConstraintCombined.

Computes, for full inputs x[8192,2048], W[2048,1000], b[1000]:
    out = x @ W + b
    lse = logsumexp(out, axis=1, keepdims=True)
    return out - (lse + log1p(-exp(out - lse)))

Math rewrite used on-device (per row):
    t = exp(out); s = sum_c t
    result = out - ln(s - t)        # == out - lse - log1p(-exp(out-lse))
(no cancellation risk: max softmax prob here is ~0.03)

Strategy:
  - Data-parallel over batch: 1024 rows per core, W/b replicated.
  - Host pre-transposes x and quantizes x,W to fp8-e4m3; the bias is
    folded in as an extra contraction row (x'=1/16 exact, W'=16*b).
  - Device runs fp8 DoubleRow matmuls (2 k-subtiles, 0.5 cyc/row) with
    one explicit ldweights per stationary tile reused by 4 C-chunks.
  - Epilogue: ACT exp(+accum) from PSUM -> bf16 t; DVE u = s - t (bf16,
    2x mode); ACT ln(u); DVE res = psum - g -> bf16; DMA out bf16.
  - Host upcasts the bf16 result to fp32.
"""
import sys

sys.path.insert(0, "/opt/trn_rl_repo")

import ml_dtypes
import numpy as np

import concourse.bass as bass
import concourse.mybir as mybir
from concourse.bass_utils import run_bass_kernel_spmd
from concourse.tile import TileContext

B, D, C = 8192, 2048, 1000
NCORES = 8
BS = B // NCORES      # 1024 rows per core
P = 128               # partitions
KO = 17               # k-subtiles: 1 bias row subtile + 16 data subtiles
KP = 8                # DoubleRow k-pairs over the 16 data subtiles
KPAD = KO * P         # 2176 padded contraction dim
MT = BS // P          # 8 m-tiles per core
CH = 250              # matmul free-dim chunk (DoubleRow moving limit 512 = 2*CH+pad)
NCH = C // CH         # 4 chunks
BIAS_X = 1.0 / 16.0   # ones-column value (exact in e4m3)
W_SCALE = 64.0        # lifts W out of fp8-subnormal range (HW flushes denormals)
INV_W_SCALE = 1.0 / W_SCALE
N_WARM = 32
F = mybir.dt.float32
F8 = mybir.dt.float8e4
BF = mybir.dt.bfloat16
AF = mybir.ActivationFunctionType
ALU = mybir.AluOpType
DR = mybir.MatmulPerfMode.DoubleRow

E4NP = ml_dtypes.float8_e4m3
BFNP = ml_dtypes.bfloat16


def _split_multi_waits(nc, max_waits=1):
    """walrus codegen on this toolchain allows a single sync-wait command per
    instruction; hoist extra waits into standalone NOPs on the same engine."""
    n = 0
    for fn in nc.m.functions:
        for bb in fn.blocks:
            new = []
            for inst in bb.instructions:
                si = inst.sync_info
                if si is not None and len(si.on_wait) > max_waits:
                    waits = list(si.on_wait)
                    for j, w in enumerate(waits[:-max_waits]):
                        nop = mybir.InstNoOp(
                            name=f"{inst.name}-w{j}", engine=inst.engine
                        )
                        nop.sync_info = mybir.SyncInfo(on_wait=[w], on_update=[])
                        new.append(nop)
                        n += 1
                    inst.sync_info = mybir.SyncInfo(
                        on_wait=waits[-max_waits:], on_update=list(si.on_update)
                    )
                new.append(inst)
            bb.instructions = new
    return n


def _body(nc, tc, xt, w, out, ctx):
    wx = ctx.enter_context(tc.tile_pool(name="wx", bufs=1))
    work = ctx.enter_context(tc.tile_pool(name="work", bufs=4))
    pso = ctx.enter_context(tc.tile_pool(name="pso", bufs=4, space="PSUM"))

    xt3 = xt.rearrange("(ko p) m -> p ko m", p=P)
    w3 = w.rearrange("(ko p) c -> p ko c", p=P)
    out2 = out.rearrange("(mt p) c -> mt p c", p=P)

    xt_sb = wx.tile([P, KO, BS], F8)
    w_sb = wx.tile([P, KO, C], F8)

    # Input strips: each DMA trigger costs ~0.7us on its issuing engine, so
    # batch into 4 ascending-size chunks per tensor and spread the triggers
    # over four otherwise-idle engines; the first chunk unblocks matmul kp0.
    DMA_SPLITS = [(0, 3), (3, 7), (7, 13), (13, KO)]
    for lo, hi in DMA_SPLITS:
        nc.gpsimd.dma_start(xt_sb[:, lo:hi, :], xt3[:, lo:hi, :])
        nc.sync.dma_start(w_sb[:, lo:hi, :], w3[:, lo:hi, :])

    # PE clock warmup on a zeroed fp8 tile while the DMAs land. Warmup
    # output goes into m-tile 0's own (not-yet-started) PSUM tile so all
    # four ring slots stay available for real accumulations.
    wtile = work.tile([P, 2, P], F8, tag="warm")
    nc.vector.memset(wtile, 0)

    ps_tiles = {}

    def bias_opener(mt):
        # Open each accumulation group with a plain-mode matmul on the bias
        # subtile: the first start=True DoubleRow write into a fresh PSUM
        # bank drops its first k-subtile on this hardware, so the group
        # opener must not be a DoubleRow op.
        msl = slice(mt * P, (mt + 1) * P)
        if mt not in ps_tiles:
            ps_tiles[mt] = pso.tile([P, 2, 512], F, tag="ps", name=f"ps_{mt}")
        ps = ps_tiles[mt]
        for ch in range(NCH):
            bk, co = divmod(ch, 2)
            nc.tensor.matmul(
                ps[:, bk, co * CH:(co + 1) * CH],
                xt_sb[:, 0, msl],
                w_sb[:, 0, ch * CH:(ch + 1) * CH],
                start=True,
                stop=False,
            )

    def kp_step(mt, kp):
        msl = slice(mt * P, (mt + 1) * P)
        ps = ps_tiles[mt]
        ksl = slice(2 * kp + 1, 2 * kp + 3)
        lhsT = xt_sb[:, ksl, msl]
        nc.tensor.ldweights(lhsT, perf_mode=DR)
        for ch in range(NCH):
            bk, co = divmod(ch, 2)
            mm = nc.tensor.matmul(
                ps[:, bk, co * CH:(co + 1) * CH],
                lhsT,
                w_sb[:, ksl, ch * CH:(ch + 1) * CH],
                start=False,
                stop=(kp == KP - 1),
                perf_mode=DR,
            )
            mm.ins.ldweights = False

    def epilogue(mt):
        # res = o - ln(s - exp(o)), all views chunk-matched
        ps = ps_tiles[mt]
        ps_v = ps[:, :, 0:2 * CH]                        # [P, 2, 500] fp32
        t = work.tile([P, C], F, tag="t", name=f"t_{mt}")
        t_v = t[:, :].rearrange("p (b c) -> p b c", b=2)
        s = work.tile([P, 1], F, tag="s", name=f"s_{mt}")
        nc.scalar.activation(t_v, ps_v, AF.Exp, scale=INV_W_SCALE, accum_out=s)
        # g = ln(s - t) folded into one ACT op via per-partition bias
        g = work.tile([P, C], F, tag="g", name=f"g_{mt}")
        nc.scalar.activation(g, t, AF.Ln, bias=s[:, :], scale=-1.0)
        res = work.tile([P, C], BF, tag="res", name=f"res_{mt}")
        res_v = res[:, :].rearrange("p (b c) -> p b c", b=2)
        g_v = g[:, :].rearrange("p (b c) -> p b c", b=2)
        nc.vector.scalar_tensor_tensor(
            res_v, ps_v, INV_W_SCALE, g_v, ALU.mult, ALU.subtract
        )
        nc.sync.dma_start(out2[mt], res)

    # Warmup matmuls into mt0's future tile (results discarded: the bias
    # opener below rewrites it with start=True).
    ps0 = ps_tiles[0] = pso.tile([P, 2, 512], F, tag="ps", name="ps_0")
    for _ in range(N_WARM):
        nc.tensor.matmul(
            ps0[:, 0, 0:P], wtile, wtile, start=True, stop=True, perf_mode=DR
        )

    # m-tiles 0-3: chunk-staged kp-major so PE always has DMA-ready work
    # while the input chunks land; staggers their completions only slightly,
    # so their epilogues interleave with m-tiles 4-7 below.
    GROUP_A = range(4)
    for mt in GROUP_A:
        bias_opener(mt)
        kp_step(mt, 0)
    for mt in GROUP_A:
        for kp in (1, 2):
            kp_step(mt, kp)
    for mt in GROUP_A:
        for kp in (3, 4, 5):
            kp_step(mt, kp)
    for mt in GROUP_A:
        kp_step(mt, 6)
        kp_step(mt, 7)
        epilogue(mt)

    # m-tiles 4-7: all inputs resident by now; straight per-tile pipeline.
    for mt in range(4, MT):
        bias_opener(mt)
        for kp in range(KP):
            kp_step(mt, kp)
        epilogue(mt)


_NC = None


def _build():
    global _NC
    if _NC is not None:
        return _NC
    nc = bass.Bass()
    xt = nc.declare_dram_parameter("xt", [KPAD, BS], F8, isOutput=False)
    w = nc.declare_dram_parameter("w", [KPAD, C], F8, isOutput=False)
    out = nc.declare_dram_parameter("out", [BS, C], BF, isOutput=True)
    from contextlib import ExitStack

    with TileContext(nc) as tc, ExitStack() as ctx:
        _body(nc, tc, xt[:, :], w[:, :], out[:, :], ctx)
    _split_multi_waits(nc)
    _NC = nc
    return nc


def kernel(x, W, b, trace=False):
    x = np.asarray(x, dtype=np.float32)
    W = np.asarray(W, dtype=np.float32)
    b = np.asarray(b, dtype=np.float32)

    # Host-side prep (not on the device critical path): transpose + fp8
    # quantize x, quantize W, fold bias in as one extra contraction row.
    xT8 = np.empty((KPAD, B), dtype=E4NP)
    xT8[0] = E4NP(BIAS_X)
    xT8[P:P + D] = np.ascontiguousarray(x.astype(E4NP).T)
    xT8[1:P] = E4NP(0.0)
    W8 = np.empty((KPAD, C), dtype=E4NP)
    W8[0] = (b * (W_SCALE / BIAS_X)).astype(E4NP)
    W8[1:P] = E4NP(0.0)
    W8[P:P + D] = (W * W_SCALE).astype(E4NP)

    nc = _build()
    in_maps = [
        {"xt": np.ascontiguousarray(xT8[:, i * BS:(i + 1) * BS]), "w": W8}
        for i in range(NCORES)
    ]
    r = run_bass_kernel_spmd(nc, in_maps, list(range(NCORES)), trace=trace)
    outp = np.concatenate(
        [r.results[i]["out"].astype(np.float32) for i in range(NCORES)], axis=0
    )
    if trace:
        return outp, r
    return outp
